# revision 8
# baseline (speedup 1.0000x reference)
"""RNNT JointNet kernel for 8 Trainium2 NeuronCores (Bass/Tile).

Math (per reference):
    enc_proj = enc @ w_enc.T          # (B,T,H)
    dec_proj = dec @ w_dec.T          # (B,U,H)
    hidden   = gelu_tanh(enc_proj[:,:,None,:] + dec_proj[:,None,:,:] + b1)
    logits   = hidden @ w2.T          # (B,T,U,V)

Sharding: 8 cores = B(4) x U-halves(2). Each core owns (b, u_half):
full T=256, U_loc=32. Weights replicated. No collectives.

Per-core dataflow (all matmuls bf16, fp32 PSUM accumulation):
  PE:  warmup spins (fire the HAM clock-gate during the load phase and
       bridge to hid/w2 readiness), then dec/enc projections, then the
       big matmul with hiddenT tiles stationary: out[t(128), v(512)] +=
       hidT[h,t_tile].T @ w2T[h,v].
  ACT: hiddenT = gelu(enc_projT + bias) where bias = dec_projT[:,u] + b1
       as a per-partition scalar -> fuses broadcast-add + bias + gelu.
  DVE: PSUM -> SBUF fp16 casts of the logits tiles.
  DMA: loads spread over all three rings in 256KB pieces, j-grouped
       weight layout so each piece unlocks one projection j-group; fp16
       stores batched as u-pairs on alternating sync/gpsimd rings (final
       u's store per-half right after each cast); host upconverts.
"""

import numpy as np

B, T, U, D = 4, 256, 64, 512
H, V = 512, 1024
P = 128
ND = D // P  # contraction-dim chunks for projections
NH = H // P  # h chunks (contraction of the big matmul)
UL = U // 2  # U per core
N_CORES = 8
WARM1_MMS = 8  # dummy N=512 matmuls to trip the HAM clock-gate early
WARM2_MMS = 4  # bridge from projections to w2/hid readiness

_CACHE = {}


def _build():
    import concourse.bass as bass  # noqa: F401
    import concourse.mybir as mybir
    from concourse import bacc, tile

    bf16 = mybir.dt.bfloat16
    f16 = mybir.dt.float16
    f32 = mybir.dt.float32
    gelu = mybir.ActivationFunctionType.Gelu_apprx_tanh

    nc = bacc.Bacc(
        "TRN2",
        target_bir_lowering=False,
        debug=False,
        enable_asserts=False,
        num_devices=N_CORES,
    )

    # Inputs arrive pre-shuffled by the host into exact SBUF images
    # ([128 partitions, free]) so every load is one contiguous DMA.
    # wdec/wenc use a j-major layout: lhsT tile (j, dc) at cols
    # j*(ND*P) + dc*P, so one 256KB piece = two complete j-groups.
    decT_d = nc.dram_tensor("decT", (P, ND * UL), bf16, kind="ExternalInput")
    encT_d = nc.dram_tensor("encT", (P, ND * T), bf16, kind="ExternalInput")
    wdecT_d = nc.dram_tensor("wdecT", (P, ND * H), bf16, kind="ExternalInput")
    wencT_d = nc.dram_tensor("wencT", (P, ND * H), bf16, kind="ExternalInput")
    w2lo_d = nc.dram_tensor("w2lo", (P, NH * 512), bf16, kind="ExternalInput")
    w2hi_d = nc.dram_tensor("w2hi", (P, NH * 512), bf16, kind="ExternalInput")
    b1c_d = nc.dram_tensor("b1c", (P, NH), f32, kind="ExternalInput")
    out_d = nc.dram_tensor("out", (T, UL, V), f16, kind="ExternalOutput")

    with tile.TileContext(nc) as tc:
        with (
            tc.tile_pool(name="const", bufs=1) as cpool,
            tc.tile_pool(name="work", bufs=1) as wpool,
            tc.tile_pool(name="hid", bufs=6) as hpool,
            tc.tile_pool(name="osb", bufs=6) as spool,
            tc.tile_pool(name="osb1", bufs=4) as s1pool,
        ):
            decT_sb = cpool.tile([P, ND * UL], bf16, tag="decT")
            encT_sb = cpool.tile([P, ND * T], bf16, tag="encT")
            wdec_sb = cpool.tile([P, ND * H], bf16, tag="wdec")
            wenc_sb = cpool.tile([P, ND * H], bf16, tag="wenc")
            w2lo_sb = cpool.tile([P, NH * 512], bf16, tag="w2lo")
            w2hi_sb = cpool.tile([P, NH * 512], bf16, tag="w2hi")
            b1_sb = cpool.tile([P, NH], f32, tag="b1")
            warm_sb = cpool.tile([P, 512], bf16, tag="warm")
            wact_sb = cpool.tile([P, 4], bf16, tag="wact")

            # ---- loads: 256KB pieces in first-use order across all rings.
            # sync: dec projection inputs, then b1 and the w2hi pieces;
            # scalar: enc projection inputs (its queue then runs the dummy
            # activation that pulls the gelu ACT tables forward);
            # gpsimd: the w2lo pieces (needed ~6us into the main loop).
            HW = ND * P  # 512 cols per j-group
            nc.sync.dma_start(out=decT_sb[:], in_=decT_d.ap()[:, :])
            nc.sync.dma_start(out=wdec_sb[:, 0:2 * HW], in_=wdecT_d.ap()[:, 0:2 * HW])
            nc.sync.dma_start(out=wdec_sb[:, 2 * HW:4 * HW], in_=wdecT_d.ap()[:, 2 * HW:4 * HW])
            nc.scalar.dma_start(out=encT_sb[:], in_=encT_d.ap()[:, :])
            nc.scalar.dma_start(out=wenc_sb[:, 0:2 * HW], in_=wencT_d.ap()[:, 0:2 * HW])
            nc.scalar.dma_start(out=wenc_sb[:, 2 * HW:4 * HW], in_=wencT_d.ap()[:, 2 * HW:4 * HW])
            nc.sync.dma_start(out=b1_sb[:], in_=b1c_d.ap()[:, :])
            nc.sync.dma_start(out=w2hi_sb[:, 0:1024], in_=w2hi_d.ap()[:, 0:1024])
            nc.sync.dma_start(out=w2hi_sb[:, 1024:2048], in_=w2hi_d.ap()[:, 1024:2048])
            nc.gpsimd.dma_start(out=w2lo_sb[:, 0:1024], in_=w2lo_d.ap()[:, 0:1024])
            nc.gpsimd.dma_start(out=w2lo_sb[:, 1024:2048], in_=w2lo_d.ap()[:, 1024:2048])

            # ---- PE warmup: dummy matmuls on a zeroed tile keep the PE busy
            # from the end of the preamble so the HAM un-throttles to 2.4GHz
            # before the real work arrives. The dummy activation pulls the
            # gelu ACT table loads forward (they are otherwise emitted lazily
            # right before the first real activation).
            nc.gpsimd.memset(warm_sb[:], 0)
            nc.scalar.activation(wact_sb[:], warm_sb[:, 0:4], gelu, bias=0.0)
            with tc.tile_pool(name="warm_ps", bufs=1, space="PSUM") as warmp:
                warm_ps = warmp.tile([P, 512], f32, tag="warm_ps")
                for _ in range(WARM1_MMS):
                    nc.tensor.matmul(
                        warm_ps[:], warm_sb[:, 0:P], warm_sb[:],
                        start=True, stop=True,
                    )

            enc_pj = wpool.tile([P, NH * T], f32, tag="enc_pj")
            dec_pj = wpool.tile([P, NH * UL], f32, tag="dec_pj")

            # ---- projections: dec_projT[h,u] first (its pieces land first),
            # then enc_projT[h,t]. Each enc j-slice gets its own PSUM bank so
            # the evacuation copy of slice j never blocks slice j+1.
            with (
                tc.tile_pool(name="decproj_ps", bufs=1, space="PSUM") as dpool,
                tc.tile_pool(name="encproj_ps", bufs=4, space="PSUM") as ppool,
            ):
                dec_ps = dpool.tile([P, NH * UL], f32, tag="dec_ps")
                for j in range(NH):  # h slice
                    for dc in range(ND):
                        lhs_cols = slice(j * HW + dc * P, j * HW + (dc + 1) * P)
                        nc.tensor.matmul(
                            dec_ps[:, j * UL:(j + 1) * UL],
                            wdec_sb[:, lhs_cols],
                            decT_sb[:, dc * UL:(dc + 1) * UL],
                            start=(dc == 0), stop=(dc == ND - 1),
                        )
                for j in range(NH):
                    nc.vector.tensor_scalar_add(
                        dec_pj[:, j * UL:(j + 1) * UL],
                        dec_ps[:, j * UL:(j + 1) * UL],
                        b1_sb[:, j:j + 1],
                    )
                for j in range(NH):
                    enc_ps = ppool.tile([P, T], f32, tag="enc_ps")
                    for dc in range(ND):
                        lhs_cols = slice(j * HW + dc * P, j * HW + (dc + 1) * P)
                        nc.tensor.matmul(
                            enc_ps[:],
                            wenc_sb[:, lhs_cols],
                            encT_sb[:, dc * T:(dc + 1) * T],
                            start=(dc == 0), stop=(dc == ND - 1),
                        )
                    # per-slice copy so gelu can start before all slices finish
                    nc.vector.tensor_copy(enc_pj[:, j * T:(j + 1) * T], enc_ps[:])

            # ---- second warmup: keep the PE busy (and the HAM warm) while
            # the w2 pieces land and ACT builds a lead producing hid tiles.
            with tc.tile_pool(name="warm2_ps", bufs=1, space="PSUM") as warmp2:
                warm2_ps = warmp2.tile([P, 512], f32, tag="warm2_ps")
                for _ in range(WARM2_MMS):
                    nc.tensor.matmul(
                        warm2_ps[:], warm_sb[:, 0:P], warm_sb[:],
                        start=True, stop=True,
                    )

            # ---- main loop over u ----
            # Stores are batched as u-pairs (one 512KB fp16 DMA per (pair,th))
            # except the final two u's, which store per 512-col half right
            # after each cast so the drain after the last matmul is short.
            osb_cur = [None, None]
            with tc.tile_pool(name="out_ps", bufs=4, space="PSUM") as opool:
                for u in range(UL):
                    hid = hpool.tile([P, NH * T], bf16, tag="hid")
                    if u == 0:
                        # th-split so chunk (i, th0) is ready in consumption
                        # order for the lo-first matmuls below
                        for th in range(T // P):
                            for i in range(NH):
                                cols = slice(i * T + th * P, i * T + (th + 1) * P)
                                nc.scalar.activation(
                                    hid[:, cols], enc_pj[:, cols], gelu,
                                    bias=dec_pj[:, i * UL: i * UL + 1],
                                )
                    else:
                        for i in range(NH):
                            nc.scalar.activation(
                                hid[:, i * T:(i + 1) * T],
                                enc_pj[:, i * T:(i + 1) * T],
                                gelu,
                                bias=dec_pj[:, i * UL + u: i * UL + u + 1],
                            )
                    ps_u = []
                    for _th in range(2):
                        po = opool.tile([P, V], f32, tag="po")
                        ps_u.append(po)
                    if u == 0:
                        # lo-half matmuls first (w2lo pieces land first)
                        for half, w2_sb in ((0, w2lo_sb), (1, w2hi_sb)):
                            for th in range(T // P):
                                for i in range(NH):
                                    lhsT = hid[:, i * T + th * P: i * T + th * P + P]
                                    nc.tensor.matmul(
                                        ps_u[th][:, half * 512:(half + 1) * 512],
                                        lhsT, w2_sb[:, i * 512:(i + 1) * 512],
                                        start=(i == 0), stop=(i == NH - 1))
                    else:
                        for th in range(T // P):
                            for i in range(NH):
                                lhsT = hid[:, i * T + th * P: i * T + th * P + P]
                                nc.tensor.matmul(
                                    ps_u[th][:, 0:512], lhsT,
                                    w2lo_sb[:, i * 512:(i + 1) * 512],
                                    start=(i == 0), stop=(i == NH - 1))
                                nc.tensor.matmul(
                                    ps_u[th][:, 512:V], lhsT,
                                    w2hi_sb[:, i * 512:(i + 1) * 512],
                                    start=(i == 0), stop=(i == NH - 1))
                    if u >= UL - 2:
                        # final u's: cast and store per 512-col half so the
                        # first half ships while the second is still casting
                        for th in range(T // P):
                            osb1 = s1pool.tile([P, V], f16, tag="osb1")
                            for half in range(2):
                                cols = slice(half * 512, (half + 1) * 512)
                                nc.vector.tensor_copy(osb1[:, cols], ps_u[th][:, cols])
                                eng = nc.sync if th == 0 else nc.scalar
                                eng.dma_start(
                                    out=out_d.ap()[th * P:(th + 1) * P, u, cols],
                                    in_=osb1[:, cols],
                                )
                    else:
                        half = u % 2
                        for th in range(T // P):
                            if half == 0:
                                osb_t = spool.tile([P, 2 * V], f16, tag="osb")
                                osb_cur[th] = osb_t
                            nc.vector.tensor_copy(
                                osb_cur[th][:, half * V:(half + 1) * V], ps_u[th][:]
                            )
                            if half == 1:
                                idx = (u // 2) * 2 + th
                                eng = nc.sync if idx % 2 == 1 else nc.gpsimd
                                eng.dma_start(
                                    out=out_d.ap()[th * P:(th + 1) * P, u - 1:u + 1, :],
                                    in_=osb_cur[th][:],
                                )

    nc.compile()
    return nc


def _get_nc():
    if "nc" not in _CACHE:
        _CACHE["nc"] = _build()
    return _CACHE["nc"]


def _sbuf_img(mat_t):
    """[R=c*128, W] -> SBUF image [128, c*W]: img[p, c*W+w] = mat_t[c*128+p, w]."""
    r, w = mat_t.shape
    c = r // P
    return np.ascontiguousarray(
        mat_t.reshape(c, P, w).transpose(1, 0, 2).reshape(P, c * w)
    )


def _w_img_jmajor(w_t):
    """[D, H] -> [128, NH*ND*128]: img[p, j*512 + dc*128 + q] = w_t[dc*128+p, j*128+q]."""
    return np.ascontiguousarray(
        w_t.reshape(ND, P, NH, P).transpose(1, 2, 0, 3).reshape(P, NH * ND * P)
    )


def _host_prep(encoder_outputs, decoder_outputs, w1, b1, w2):
    import ml_dtypes

    bf16 = ml_dtypes.bfloat16
    w_encT = _w_img_jmajor(w1[:, :D].T.astype(bf16))   # [D,H] -> j-major image
    w_decT = _w_img_jmajor(w1[:, D:].T.astype(bf16))
    w2T = w2.T.astype(bf16)                             # [H, V]
    w2lo = _sbuf_img(w2T[:, 0:512])                     # [128, NH*512]
    w2hi = _sbuf_img(w2T[:, 512:V])
    b1c = np.ascontiguousarray(b1.reshape(NH, P).T).astype(np.float32)
    in_maps = []
    for c in range(N_CORES):
        b, uh = divmod(c, 2)
        encT = _sbuf_img(encoder_outputs[b].T.astype(bf16))  # [D,T] -> [128, ND*T]
        decT = _sbuf_img(
            decoder_outputs[b, uh * UL:(uh + 1) * UL, :].T.astype(bf16)
        )
        in_maps.append({
            "decT": decT,
            "encT": encT,
            "wdecT": w_decT,
            "wencT": w_encT,
            "w2lo": w2lo,
            "w2hi": w2hi,
            "b1c": b1c,
        })
    return in_maps


def _gather(results):
    out = np.empty((B, T, U, V), dtype=np.float32)
    for c in range(N_CORES):
        b, uh = divmod(c, 2)
        out[b, :, uh * UL:(uh + 1) * UL, :] = np.asarray(
            results[c]["out"], dtype=np.float32
        )
    return out


def kernel(encoder_outputs, decoder_outputs, w1, b1, w2):
    from concourse import bass_utils

    nc = _get_nc()
    in_maps = _host_prep(
        np.asarray(encoder_outputs), np.asarray(decoder_outputs),
        np.asarray(w1), np.asarray(b1), np.asarray(w2),
    )
    res = bass_utils.run_bass_kernel_spmd(nc, in_maps, core_ids=list(range(N_CORES)))
    return _gather(res.results)


# revision 12
# speedup vs baseline: 1.0292x; 1.0292x over previous
"""RNNT JointNet kernel for 8 Trainium2 NeuronCores (Bass/Tile).

Math (per reference):
    enc_proj = enc @ w_enc.T          # (B,T,H)
    dec_proj = dec @ w_dec.T          # (B,U,H)
    hidden   = gelu_tanh(enc_proj[:,:,None,:] + dec_proj[:,None,:,:] + b1)
    logits   = hidden @ w2.T          # (B,T,U,V)

Sharding: 8 cores = B(4) x U-halves(2). Each core owns (b, u_half):
full T=256, U_loc=32. Weights replicated. No collectives.

Per-core dataflow (all matmuls bf16, fp32 PSUM accumulation):
  PE:  warmup spins (fire the HAM clock-gate during the load phase and
       bridge to hid/w2 readiness), then dec/enc projections, then the
       big matmul with hiddenT tiles stationary: out[t(128), v(512)] +=
       hidT[h,t_tile].T @ w2T[h,v].
  ACT: hiddenT = gelu(enc_projT + bias) where bias = dec_projT[:,u] + b1
       as a per-partition scalar -> fuses broadcast-add + bias + gelu.
  DVE: PSUM -> SBUF fp16 casts of the logits tiles.
  DMA: loads spread over all three rings in 256KB pieces, j-grouped
       weight layout so each piece unlocks one projection j-group; fp16
       stores batched as u-pairs on alternating sync/gpsimd rings (final
       u's store per-half right after each cast); host upconverts.
"""

import numpy as np

B, T, U, D = 4, 256, 64, 512
H, V = 512, 1024
P = 128
ND = D // P  # contraction-dim chunks for projections
NH = H // P  # h chunks (contraction of the big matmul)
UL = U // 2  # U per core
N_CORES = 8
WARM1_MMS = 14  # dummy N=512 matmuls: trip the HAM clock-gate early and
                # bridge the PE to load-semaphore readiness with no idle gap
WARM2_MMS = 4   # bridge from projections to w2/hid readiness

_CACHE = {}


def _build():
    import concourse.bass as bass  # noqa: F401
    import concourse.mybir as mybir
    from concourse import bacc, tile

    bf16 = mybir.dt.bfloat16
    f16 = mybir.dt.float16
    f32 = mybir.dt.float32
    gelu = mybir.ActivationFunctionType.Gelu_apprx_tanh

    nc = bacc.Bacc(
        "TRN2",
        target_bir_lowering=False,
        debug=False,
        enable_asserts=False,
        num_devices=N_CORES,
    )

    # Inputs arrive pre-shuffled by the host into exact SBUF images
    # ([128 partitions, free]) so every load is one contiguous DMA.
    # pack_a = [decT | wdecT], pack_b = [encT | wencT]; wdec/wenc use a
    # j-major layout: lhsT tile (j, dc) at cols j*(ND*P) + dc*P.
    PA = ND * UL + ND * H      # 128 + 2048
    PB = ND * T + ND * H       # 1024 + 2048
    packa_d = nc.dram_tensor("packa", (P, PA), bf16, kind="ExternalInput")
    packb_d = nc.dram_tensor("packb", (P, PB), bf16, kind="ExternalInput")
    w2lo_d = nc.dram_tensor("w2lo", (P, NH * 512), bf16, kind="ExternalInput")
    w2hi_d = nc.dram_tensor("w2hi", (P, NH * 512), bf16, kind="ExternalInput")
    b1c_d = nc.dram_tensor("b1c", (P, NH), f32, kind="ExternalInput")
    out_d = nc.dram_tensor("out", (T, UL, V), f16, kind="ExternalOutput")

    with tile.TileContext(nc) as tc:
        with (
            tc.tile_pool(name="const", bufs=1) as cpool,
            tc.tile_pool(name="work", bufs=1) as wpool,
            tc.tile_pool(name="hid", bufs=6) as hpool,
            tc.tile_pool(name="osb", bufs=6) as spool,
            tc.tile_pool(name="osb1", bufs=4) as s1pool,
        ):
            packa_sb = cpool.tile([P, PA], bf16, tag="packa")
            packb_sb = cpool.tile([P, PB], bf16, tag="packb")
            w2lo_sb = cpool.tile([P, NH * 512], bf16, tag="w2lo")
            w2hi_sb = cpool.tile([P, NH * 512], bf16, tag="w2hi")
            b1_sb = cpool.tile([P, NH], f32, tag="b1")
            warm_sb = cpool.tile([P, 512], bf16, tag="warm")
            wact_sb = cpool.tile([P, 4], bf16, tag="wact")

            decT_sb = packa_sb[:, 0:ND * UL]
            wdec_sb = packa_sb[:, ND * UL:PA]
            encT_sb = packb_sb[:, 0:ND * T]
            wenc_sb = packb_sb[:, ND * T:PB]

            # ---- loads: split across the two HWDGE rings, first-use order,
            # w2 halves in 256KB pieces so the first tile's matmuls can start
            # as the pieces land. No SWDGE traffic in the load phase.
            HW = ND * P  # 512 cols per j-group in the j-major weight images
            nc.sync.dma_start(out=b1_sb[:], in_=b1c_d.ap()[:, :])
            nc.sync.dma_start(out=packa_sb[:], in_=packa_d.ap()[:, :])
            nc.scalar.dma_start(out=packb_sb[:], in_=packb_d.ap()[:, :])
            nc.sync.dma_start(out=w2lo_sb[:, 0:1024], in_=w2lo_d.ap()[:, 0:1024])
            nc.sync.dma_start(out=w2lo_sb[:, 1024:2048], in_=w2lo_d.ap()[:, 1024:2048])
            nc.scalar.dma_start(out=w2hi_sb[:, 0:1024], in_=w2hi_d.ap()[:, 0:1024])
            nc.scalar.dma_start(out=w2hi_sb[:, 1024:2048], in_=w2hi_d.ap()[:, 1024:2048])

            # ---- PE warmup: dummy matmuls on a zeroed tile keep the PE busy
            # from the end of the preamble so the HAM un-throttles to 2.4GHz
            # and the PE reaches the load-gated work with no idle gap. The
            # dummy activation pulls the gelu ACT table loads forward (they
            # are otherwise emitted lazily before the first real activation).
            nc.vector.memset(warm_sb[:], 0)
            nc.scalar.activation(wact_sb[:], warm_sb[:, 0:4], gelu, bias=0.0)
            with tc.tile_pool(name="warm_ps", bufs=1, space="PSUM") as warmp:
                warm_ps = warmp.tile([P, 512], f32, tag="warm_ps")
                for _ in range(WARM1_MMS):
                    nc.tensor.matmul(
                        warm_ps[:], warm_sb[:, 0:P], warm_sb[:],
                        start=True, stop=True,
                    )

            enc_pj = wpool.tile([P, NH * T], f32, tag="enc_pj")
            dec_pj = wpool.tile([P, NH * UL], f32, tag="dec_pj")

            # ---- projections: dec_projT[h,u] first (its pieces land first),
            # then enc_projT[h,t]. Each enc j-slice gets its own PSUM bank so
            # the evacuation copy of slice j never blocks slice j+1.
            with (
                tc.tile_pool(name="decproj_ps", bufs=1, space="PSUM") as dpool,
                tc.tile_pool(name="encproj_ps", bufs=4, space="PSUM") as ppool,
            ):
                dec_ps = dpool.tile([P, NH * UL], f32, tag="dec_ps")
                for j in range(NH):  # h slice
                    for dc in range(ND):
                        lhs_cols = slice(j * HW + dc * P, j * HW + (dc + 1) * P)
                        nc.tensor.matmul(
                            dec_ps[:, j * UL:(j + 1) * UL],
                            wdec_sb[:, lhs_cols],
                            decT_sb[:, dc * UL:(dc + 1) * UL],
                            start=(dc == 0), stop=(dc == ND - 1),
                        )
                for j in range(NH):
                    nc.vector.tensor_scalar_add(
                        dec_pj[:, j * UL:(j + 1) * UL],
                        dec_ps[:, j * UL:(j + 1) * UL],
                        b1_sb[:, j:j + 1],
                    )
                for j in range(NH):
                    enc_ps = ppool.tile([P, T], f32, tag="enc_ps")
                    for dc in range(ND):
                        lhs_cols = slice(j * HW + dc * P, j * HW + (dc + 1) * P)
                        nc.tensor.matmul(
                            enc_ps[:],
                            wenc_sb[:, lhs_cols],
                            encT_sb[:, dc * T:(dc + 1) * T],
                            start=(dc == 0), stop=(dc == ND - 1),
                        )
                    # per-slice copy so gelu can start before all slices finish
                    nc.vector.tensor_copy(enc_pj[:, j * T:(j + 1) * T], enc_ps[:])

            # ---- second warmup: keep the PE busy (and the HAM warm) while
            # the w2 pieces land and ACT builds a lead producing hid tiles.
            with tc.tile_pool(name="warm2_ps", bufs=1, space="PSUM") as warmp2:
                warm2_ps = warmp2.tile([P, 512], f32, tag="warm2_ps")
                for _ in range(WARM2_MMS):
                    nc.tensor.matmul(
                        warm2_ps[:], warm_sb[:, 0:P], warm_sb[:],
                        start=True, stop=True,
                    )

            # ---- main loop over u ----
            # Stores are batched as u-pairs (one 512KB fp16 DMA per (pair,th))
            # except the final two u's, which store per 512-col half right
            # after each cast so the drain after the last matmul is short.
            osb_cur = [None, None]
            with tc.tile_pool(name="out_ps", bufs=4, space="PSUM") as opool:
                for u in range(UL):
                    hid = hpool.tile([P, NH * T], bf16, tag="hid")
                    if u == 0:
                        # th-split so chunk (i, th0) is ready in consumption
                        # order for the lo-first matmuls below
                        for th in range(T // P):
                            for i in range(NH):
                                cols = slice(i * T + th * P, i * T + (th + 1) * P)
                                nc.scalar.activation(
                                    hid[:, cols], enc_pj[:, cols], gelu,
                                    bias=dec_pj[:, i * UL: i * UL + 1],
                                )
                    else:
                        for i in range(NH):
                            nc.scalar.activation(
                                hid[:, i * T:(i + 1) * T],
                                enc_pj[:, i * T:(i + 1) * T],
                                gelu,
                                bias=dec_pj[:, i * UL + u: i * UL + u + 1],
                            )
                    ps_u = []
                    for _th in range(2):
                        po = opool.tile([P, V], f32, tag="po")
                        ps_u.append(po)
                    if u == 0:
                        # lo-half matmuls first (w2lo pieces land first)
                        for half, w2_sb in ((0, w2lo_sb), (1, w2hi_sb)):
                            for th in range(T // P):
                                for i in range(NH):
                                    lhsT = hid[:, i * T + th * P: i * T + th * P + P]
                                    nc.tensor.matmul(
                                        ps_u[th][:, half * 512:(half + 1) * 512],
                                        lhsT, w2_sb[:, i * 512:(i + 1) * 512],
                                        start=(i == 0), stop=(i == NH - 1))
                    else:
                        for th in range(T // P):
                            for i in range(NH):
                                lhsT = hid[:, i * T + th * P: i * T + th * P + P]
                                nc.tensor.matmul(
                                    ps_u[th][:, 0:512], lhsT,
                                    w2lo_sb[:, i * 512:(i + 1) * 512],
                                    start=(i == 0), stop=(i == NH - 1))
                                nc.tensor.matmul(
                                    ps_u[th][:, 512:V], lhsT,
                                    w2hi_sb[:, i * 512:(i + 1) * 512],
                                    start=(i == 0), stop=(i == NH - 1))
                    if u >= UL - 2:
                        # final u's: cast and store per 512-col half so the
                        # first half ships while the second is still casting
                        for th in range(T // P):
                            osb1 = s1pool.tile([P, V], f16, tag="osb1")
                            for half in range(2):
                                cols = slice(half * 512, (half + 1) * 512)
                                nc.vector.tensor_copy(osb1[:, cols], ps_u[th][:, cols])
                                eng = nc.sync if th == 0 else nc.scalar
                                eng.dma_start(
                                    out=out_d.ap()[th * P:(th + 1) * P, u, cols],
                                    in_=osb1[:, cols],
                                )
                    else:
                        half = u % 2
                        for th in range(T // P):
                            if half == 0:
                                osb_t = spool.tile([P, 2 * V], f16, tag="osb")
                                osb_cur[th] = osb_t
                            nc.vector.tensor_copy(
                                osb_cur[th][:, half * V:(half + 1) * V], ps_u[th][:]
                            )
                            if half == 1:
                                idx = (u // 2) * 2 + th
                                eng = nc.sync if idx % 2 == 1 else nc.gpsimd
                                eng.dma_start(
                                    out=out_d.ap()[th * P:(th + 1) * P, u - 1:u + 1, :],
                                    in_=osb_cur[th][:],
                                )

    nc.compile()
    return nc


def _get_nc():
    if "nc" not in _CACHE:
        _CACHE["nc"] = _build()
    return _CACHE["nc"]


def _sbuf_img(mat_t):
    """[R=c*128, W] -> SBUF image [128, c*W]: img[p, c*W+w] = mat_t[c*128+p, w]."""
    r, w = mat_t.shape
    c = r // P
    return np.ascontiguousarray(
        mat_t.reshape(c, P, w).transpose(1, 0, 2).reshape(P, c * w)
    )


def _w_img_jmajor(w_t):
    """[D, H] -> [128, NH*ND*128]: img[p, j*512 + dc*128 + q] = w_t[dc*128+p, j*128+q]."""
    return np.ascontiguousarray(
        w_t.reshape(ND, P, NH, P).transpose(1, 2, 0, 3).reshape(P, NH * ND * P)
    )


def _host_prep(encoder_outputs, decoder_outputs, w1, b1, w2):
    import ml_dtypes

    bf16 = ml_dtypes.bfloat16
    w_encT = _w_img_jmajor(w1[:, :D].T.astype(bf16))   # [D,H] -> j-major image
    w_decT = _w_img_jmajor(w1[:, D:].T.astype(bf16))
    w2T = w2.T.astype(bf16)                             # [H, V]
    w2lo = _sbuf_img(w2T[:, 0:512])                     # [128, NH*512]
    w2hi = _sbuf_img(w2T[:, 512:V])
    b1c = np.ascontiguousarray(b1.reshape(NH, P).T).astype(np.float32)
    in_maps = []
    for c in range(N_CORES):
        b, uh = divmod(c, 2)
        encT = _sbuf_img(encoder_outputs[b].T.astype(bf16))  # [D,T] -> [128, ND*T]
        decT = _sbuf_img(
            decoder_outputs[b, uh * UL:(uh + 1) * UL, :].T.astype(bf16)
        )
        packa = np.concatenate([decT, w_decT], axis=1)
        packb = np.concatenate([encT, w_encT], axis=1)
        in_maps.append({
            "packa": np.ascontiguousarray(packa),
            "packb": np.ascontiguousarray(packb),
            "w2lo": w2lo,
            "w2hi": w2hi,
            "b1c": b1c,
        })
    return in_maps


def _gather(results):
    out = np.empty((B, T, U, V), dtype=np.float32)
    for c in range(N_CORES):
        b, uh = divmod(c, 2)
        out[b, :, uh * UL:(uh + 1) * UL, :] = np.asarray(
            results[c]["out"], dtype=np.float32
        )
    return out


def kernel(encoder_outputs, decoder_outputs, w1, b1, w2):
    from concourse import bass_utils

    nc = _get_nc()
    in_maps = _host_prep(
        np.asarray(encoder_outputs), np.asarray(decoder_outputs),
        np.asarray(w1), np.asarray(b1), np.asarray(w2),
    )
    res = bass_utils.run_bass_kernel_spmd(nc, in_maps, core_ids=list(range(N_CORES)))
    return _gather(res.results)
